# revision 1
# baseline (speedup 1.0000x reference)
"""Trainium2 Bass kernel for 2-layer GAT (nn_GAT_84146999263862).

Strategy (8 NeuronCores, SPMD):
  - Nodes padded to NP=50176 = 8*49*128; core c owns node slice [c*6272,(c+1)*6272).
  - Edges (plus self-loops) are assigned to cores by dst slice, grouped per
    128-node dst block, packed into 128-edge chunks (dummy pad edges get
    dst_local=255 so their one-hot column is all-zero).
  - Per-edge rows are fetched with gpsimd.dma_gather (int16 indices => tables
    split lo/hi at row 32768; chunks grouped by src half; per-core counts are
    padded to a shared static graph).
  - Per chunk: is_equal one-hot matrices map edges<->dst slots; PE matmuls do
    adst expansion and segment reduction (messages + softmax denominators
    accumulate in PSUM per 128-node block). Softmax skips max-subtraction
    (logits are O(1); mathematically identical).
  - Each core builds its z1 table locally in ROTATED node order (own nodes at
    rows 0..SLICE) so adst/h1 addresses are static; gather indices compensate.
  - One small AllGather shares z2_ext between layers.

Host does integer-only preprocessing (sorting/packing/index maps and pure data
movement like transposes); all floating-point math runs on device.
"""
import sys
import numpy as np

sys.path.insert(0, '/opt/trn_rl_repo')

from contextlib import ExitStack
from concourse import bass, bacc, mybir, tile, library_config
from concourse.bass_utils import run_bass_kernel_spmd
from concourse.masks import make_identity
from concourse.tile import ScopedClock

# This walrus build rejects multi-wait TPB_CTRL instructions; split the Tile
# tail-drain's semaphore waits across single-wait drains.
_MAXW = 1


def _patched_drain_and_barrier(self, tick_clock, wait_clock):
    drain = self.nc.sync.drain()
    wait_clock.add_sem_waits(drain.ins,
                             ScopedClock({None: tick_clock.global_clock}))
    si = drain.ins.sync_info
    waits = list(si.on_wait)
    if len(waits) > _MAXW:
        si.on_wait = waits[:_MAXW]
        for k in range(_MAXW, len(waits), _MAXW):
            extra = self.nc.sync.drain()
            extra.ins.sync_info = mybir.SyncInfo(on_wait=waits[k:k+_MAXW],
                                                 on_update=[])
    self.nc.all_engine_barrier()
    popped = self.nc._tile_sem_poison_stack.pop()
    assert popped is self._sem_poison
    self.nc.clear_and_free_semaphores(list(self.sems.allocated().values()))
    self.nc.all_engine_barrier()


tile.TileContext._drain_and_barrier = _patched_drain_and_barrier

F32 = mybir.dt.float32
I16 = mybir.dt.int16
NEG = 0.2
BLK = 128
GG = 8                     # chunks per dma_gather instruction (1024 indices)
MG = 4                     # chunks per metarow-broadcast matmul (512 cols)


class Cfg:
    def __init__(self, N=50000, cores=8, bpc=49, half=32768,
                 f_in=256, heads=8, ch=32, cls_=32):
        self.N = N
        self.CORES = cores
        self.BPC = bpc
        self.SLICE = bpc * BLK
        self.NP = cores * self.SLICE
        self.HALF = half
        self.F_IN = f_in
        self.HEADS = heads
        self.CH = ch
        self.D1 = heads * ch
        self.CLS = cls_
        self.TAB1_W = 320 if self.D1 == 256 else self.D1 + 64   # row: z|asrc|adst|pad
        self.TAB2_W = 64                                        # z2|asrc2|adst2|pad
        assert self.TAB1_W * 4 % 256 == 0 and self.TAB2_W * 4 % 256 == 0
        assert self.HALF % BLK == 0 and self.HALF < 32768 + 1
        assert self.NP - self.HALF <= 32767


FULL = Cfg()


# ---------------------------------------------------------------- host side

def _wrap16(vals):
    """[1024] ints -> [128, 64] int16 (wrapped 16 partitions, replicated x8)."""
    v = np.asarray(vals, np.int64).reshape(64, 16)
    arr = np.zeros((128, 64), np.int16)
    arr[:16, :] = v.T
    for r in range(1, 8):
        arr[r*16:(r+1)*16] = arr[:16]
    return arr


def host_prep(edge_index, cfg):
    """Integer-only preprocessing. Returns (counts, per_core_arrays)."""
    src = np.asarray(edge_index[0], np.int64)
    dst = np.asarray(edge_index[1], np.int64)
    loops = np.arange(cfg.N, dtype=np.int64)
    src = np.concatenate([src, loops])
    dst = np.concatenate([dst, loops])

    core = dst // cfg.SLICE
    blk_in_core = (dst % cfg.SLICE) // BLK
    dst_local = dst % BLK

    def chunkify(iv, dl):
        out = []
        for i in range(0, len(iv), BLK):
            a, b = iv[i:i+BLK], dl[i:i+BLK]
            pad = BLK - len(a)
            if pad:
                a = np.concatenate([a, np.zeros(pad, np.int64)])
                b = np.concatenate([b, np.full(pad, 255, np.int64)])
            out.append((a, b))
        return out

    ch = {1: {}, 2: {}}
    for c in range(cfg.CORES):
        m_c = core == c
        s_c, dl_c, bi_c = src[m_c], dst_local[m_c], blk_in_core[m_c]
        rot = (s_c - cfg.SLICE * c) % cfg.NP
        for layer, ids in ((1, rot), (2, s_c)):
            lo = ids < cfg.HALF
            for i in range(cfg.BPC):
                m_b = bi_c == i
                for grp in range(2):
                    m = m_b & (lo if grp == 0 else ~lo)
                    iv = ids[m] - (0 if grp == 0 else cfg.HALF)
                    ch[layer][(c, i, grp)] = chunkify(iv, dl_c[m])

    counts = {}
    for layer in (1, 2):
        nmax = np.zeros((cfg.BPC, 2), np.int64)
        for (c, i, g), lst in ch[layer].items():
            nmax[i, g] = max(nmax[i, g], len(lst))
        counts[layer] = nmax

    per_core = []
    for c in range(cfg.CORES):
        data = {}
        for layer in (1, 2):
            nmax = counts[layer]
            all_chunks = []
            for i in range(cfg.BPC):
                for g in range(2):
                    lst = ch[layer][(c, i, g)]
                    for k in range(int(nmax[i, g])):
                        if k < len(lst):
                            iv, dl = lst[k]
                        else:
                            iv = np.zeros(BLK, np.int64)
                            dl = np.full(BLK, 255, np.int64)
                        all_chunks.append((g, iv, dl))
            totch = len(all_chunks)
            stream = [[], []]
            for g, iv, dl in all_chunks:
                stream[g].append(iv)
            for g in range(2):
                s = stream[g]
                while len(s) % GG:
                    s.append(np.zeros(BLK, np.int64))
                ng = max(1, len(s) // GG)
                arr = np.zeros((128, ng * 64), np.int16)
                for gi in range(len(s) // GG):
                    arr[:, gi*64:(gi+1)*64] = _wrap16(
                        np.concatenate(s[gi*GG:(gi+1)*GG]))
                data[('idxlo' if g == 0 else 'idxhi') + str(layer)] = arr
            metacol = np.zeros((128, totch), np.float32)
            metarow = np.zeros((1, totch * BLK), np.float32)
            for j, (g, iv, dl) in enumerate(all_chunks):
                metacol[:, j] = dl
                metarow[0, j*BLK:(j+1)*BLK] = dl
            data[f'metacol{layer}'] = metacol
            data[f'metarow{layer}'] = metarow
        per_core.append(data)
    return counts, per_core


def host_weights(inputs, cfg):
    """Weight/constant staging (reordering + transposes only, no math)."""
    W1 = np.asarray(inputs['W1'], np.float32)
    a_src1 = np.asarray(inputs['a_src1'], np.float32)
    a_dst1 = np.asarray(inputs['a_dst1'], np.float32)
    b1 = np.asarray(inputs['b1'], np.float32)
    W2 = np.asarray(inputs['W2'], np.float32)
    a_src2 = np.asarray(inputs['a_src2'], np.float32)
    a_dst2 = np.asarray(inputs['a_dst2'], np.float32)
    b2 = np.asarray(inputs['b2'], np.float32)

    H, C, D1 = cfg.HEADS, cfg.CH, cfg.D1
    perm = np.empty(D1, np.int64)
    for h in range(H):
        for c_ in range(C):
            perm[c_*H + h] = h*C + c_
    consts = {
        'W1cm': W1[:, perm].copy(), 'W1T': W1.T.copy(),
        'a_src1': a_src1, 'a_dst1': a_dst1, 'b1cm': b1[perm][None, :].copy(),
        'W2p': W2[perm, :].copy(), 'W2pT': W2[perm, :].T.copy(),
        'a_src2': a_src2, 'a_dst2': a_dst2, 'b2': b2[None, :].copy(),
        'iota_row': np.broadcast_to(np.arange(128, dtype=np.float32),
                                    (128, 128)).copy(),
        'iota_col': np.arange(128, dtype=np.float32)[:, None].copy(),
        'ones_row': np.ones((1, 128), np.float32),
    }
    x = np.asarray(inputs['x'], np.float32)
    xpad = np.zeros((cfg.NP, cfg.F_IN), np.float32)
    xpad[:cfg.N] = x
    xT_rots = [np.roll(xpad, -cfg.SLICE * c, axis=0).T.copy()
               for c in range(cfg.CORES)]
    return consts, xT_rots


# ---------------------------------------------------------------- device side

def build_gat(counts, cfg):
    nc = bacc.Bacc()
    H, C, D1, CLS, F_IN = cfg.HEADS, cfg.CH, cfg.D1, cfg.CLS, cfg.F_IN
    T1, T2 = cfg.TAB1_W, cfg.TAB2_W
    E1 = D1 + 2 * H          # written z1 table cols (z | asrc | adst)
    FH = F_IN // 128

    def n_stream(layer, g):
        return max(1, -(-int(counts[layer][:, g].sum()) // GG))

    GLO1, GHI1 = n_stream(1, 0), n_stream(1, 1)
    GLO2, GHI2 = n_stream(2, 0), n_stream(2, 1)
    TOT1, TOT2 = int(counts[1].sum()), int(counts[2].sum())

    inp = {}
    for name, shape, dt in [
        ('xT_rot', [F_IN, cfg.NP], F32),
        ('W1cm', [F_IN, D1], F32), ('W1T', [D1, F_IN], F32),
        ('a_src1', [H, C], F32), ('a_dst1', [H, C], F32),
        ('b1cm', [1, D1], F32),
        ('W2p', [D1, CLS], F32), ('W2pT', [CLS, D1], F32),
        ('a_src2', [1, CLS], F32), ('a_dst2', [1, CLS], F32),
        ('b2', [1, CLS], F32),
        ('iota_row', [128, 128], F32), ('iota_col', [128, 1], F32),
        ('ones_row', [1, 128], F32),
        ('idxlo1', [128, GLO1 * 64], I16), ('idxhi1', [128, GHI1 * 64], I16),
        ('idxlo2', [128, GLO2 * 64], I16), ('idxhi2', [128, GHI2 * 64], I16),
        ('metacol1', [128, TOT1], F32), ('metarow1', [1, TOT1 * BLK], F32),
        ('metacol2', [128, TOT2], F32), ('metarow2', [1, TOT2 * BLK], F32),
    ]:
        inp[name] = nc.declare_dram_parameter(name, shape, dt, isOutput=False)

    out_d = nc.declare_dram_parameter('out', [cfg.SLICE, CLS], F32, isOutput=True)

    z1tab = nc.dram_tensor('z1tab', [cfg.NP, T1], F32)
    h1loc = nc.dram_tensor('h1loc', [cfg.SLICE, D1], F32)
    z2slice = nc.dram_tensor('z2slice', [cfg.SLICE, T2], F32)
    z2cat = nc.dram_tensor('z2cat', [cfg.NP, T2], F32)

    with tile.TileContext(nc) as tc, ExitStack() as ctx:
        sb = ctx.enter_context(tc.tile_pool(name='sb', bufs=1))
        sbw = ctx.enter_context(tc.tile_pool(name='sbw', bufs=2))

        nc.gpsimd.load_library(library_config.mlp)

        ident = sb.tile([128, 128], F32)
        make_identity(nc, ident[:])
        iota_r = sb.tile([128, 128], F32)
        nc.sync.dma_start(out=iota_r[:], in_=inp['iota_row'][:, :])
        iota_c = sb.tile([128, 1], F32)
        nc.sync.dma_start(out=iota_c[:], in_=inp['iota_col'][:, :])
        ones_r = sb.tile([1, 128], F32)
        nc.sync.dma_start(out=ones_r[:], in_=inp['ones_row'][:, :])

        W1e = [sb.tile([128, T1], F32, tag=f'w1e{_i}', name=f'W1e{_i}') for _i in range(FH)]
        W2e = [sb.tile([128, T2], F32, tag=f'w2e{_i}', name=f'W2e{_i}') for _i in range(FH)]
        b1b = sb.tile([128, D1], F32)
        b2b = sb.tile([128, CLS], F32)

        with tc.tile_pool(name='p0sb', bufs=1) as p0sb, \
             tc.tile_pool(name='p0ps', bufs=1, space='PSUM') as p0ps:
            for fh in range(FH):
                nc.vector.memset(W1e[fh][:], 0.0)
                nc.vector.memset(W2e[fh][:], 0.0)
            # ---- W1_ext = [W1cm | W1@A_src | W1@A_dst]
            a1 = p0sb.tile([H, 2 * C], F32)
            nc.sync.dma_start(out=a1[:, 0:C], in_=inp['a_src1'][:, :])
            nc.sync.dma_start(out=a1[:, C:2*C], in_=inp['a_dst1'][:, :])
            a1T_ps = p0ps.tile([128, 128], F32, space='PSUM', tag='t')
            nc.tensor.transpose(out=a1T_ps[0:2*C, 0:H], in_=a1[:], identity=ident[0:H, 0:H])
            a1T = p0sb.tile([2 * C, H], F32)
            nc.vector.tensor_copy(out=a1T[:], in_=a1T_ps[0:2*C, 0:H])
            A_bd = p0sb.tile([128, FH, 2 * H], F32)
            nc.vector.memset(A_bd[:], 0.0)
            for h in range(H):
                half, off = divmod(h * C, 128)
                nc.vector.tensor_copy(out=A_bd[off:off+C, half, h:h+1],
                                      in_=a1T[0:C, h:h+1])
                nc.vector.tensor_copy(out=A_bd[off:off+C, half, H+h:H+h+1],
                                      in_=a1T[C:2*C, h:h+1])
            w1t_sb = [p0sb.tile([128, F_IN], F32, tag=f'w1t{_i}', name=f'w1t{_i}') for _i in range(FH)]
            for cc in range(FH):
                nc.sync.dma_start(out=w1t_sb[cc][:],
                                  in_=inp['W1T'][cc*128:(cc+1)*128, :])
            w1a_ps = p0ps.tile([128, FH, 2 * H], F32, space='PSUM', tag='a')
            for fh in range(FH):
                for cc in range(FH):
                    nc.tensor.matmul(out=w1a_ps[:, fh, :],
                                     lhsT=w1t_sb[cc][:, fh*128:(fh+1)*128],
                                     rhs=A_bd[:, cc, :],
                                     start=(cc == 0), stop=(cc == FH - 1))
            for fh in range(FH):
                nc.sync.dma_start(out=W1e[fh][:, 0:D1],
                                  in_=inp['W1cm'][fh*128:(fh+1)*128, :])
                nc.vector.tensor_copy(out=W1e[fh][:, D1:D1+2*H],
                                      in_=w1a_ps[:, fh, :])

            # ---- W2_ext = [W2p | W2p@a_src2^T | W2p@a_dst2^T]
            a2 = p0sb.tile([2, CLS], F32)
            nc.sync.dma_start(out=a2[0:1, :], in_=inp['a_src2'][:, :])
            nc.sync.dma_start(out=a2[1:2, :], in_=inp['a_dst2'][:, :])
            a2T_ps = p0ps.tile([128, 128], F32, space='PSUM', tag='t')
            nc.tensor.transpose(out=a2T_ps[0:CLS, 0:2], in_=a2[:], identity=ident[0:2, 0:2])
            a2T = p0sb.tile([CLS, 2], F32)
            nc.vector.tensor_copy(out=a2T[:], in_=a2T_ps[0:CLS, 0:2])
            w2t_sb = p0sb.tile([CLS, D1], F32)
            nc.sync.dma_start(out=w2t_sb[:], in_=inp['W2pT'][:, :])
            w2a_ps = p0ps.tile([128, FH, 2], F32, space='PSUM', tag='a')
            for fh in range(FH):
                nc.tensor.matmul(out=w2a_ps[:, fh, :],
                                 lhsT=w2t_sb[:, fh*128:(fh+1)*128],
                                 rhs=a2T[:], start=True, stop=True)
            for fh in range(FH):
                nc.sync.dma_start(out=W2e[fh][:, 0:CLS],
                                  in_=inp['W2p'][fh*128:(fh+1)*128, :])
                nc.vector.tensor_copy(out=W2e[fh][:, CLS:CLS+2],
                                      in_=w2a_ps[:, fh, :])

            # ---- bias broadcast tiles
            b1_sb = p0sb.tile([1, D1], F32)
            nc.sync.dma_start(out=b1_sb[:], in_=inp['b1cm'][:, :])
            b1b_ps = p0ps.tile([128, D1], F32, space='PSUM', tag='b')
            nc.tensor.matmul(out=b1b_ps[:], lhsT=ones_r[:], rhs=b1_sb[:],
                             start=True, stop=True)
            nc.vector.tensor_copy(out=b1b[:], in_=b1b_ps[:])
            b2_sb = p0sb.tile([1, CLS], F32)
            nc.sync.dma_start(out=b2_sb[:], in_=inp['b2'][:, :])
            b2b_ps = p0ps.tile([128, CLS], F32, space='PSUM', tag='b')
            nc.tensor.matmul(out=b2b_ps[:], lhsT=ones_r[:], rhs=b2_sb[:],
                             start=True, stop=True)
            nc.vector.tensor_copy(out=b2b[:], in_=b2b_ps[:])

        # ---- P1: z1 table build (rotated order)
        with tc.tile_pool(name='p1sb', bufs=3) as p1sb, \
             tc.tile_pool(name='p1ps', bufs=2, space='PSUM') as p1ps:
            for t in range(cfg.NP // 128):
                zps = p1ps.tile([128, T1], F32, space='PSUM')
                for fh in range(FH):
                    xt = p1sb.tile([128, 128], F32, tag='xt')
                    nc.sync.dma_start(
                        out=xt[:],
                        in_=inp['xT_rot'][fh*128:(fh+1)*128, t*128:(t+1)*128])
                    nc.tensor.matmul(out=zps[:], lhsT=xt[:], rhs=W1e[fh][:],
                                     start=(fh == 0), stop=(fh == FH - 1))
                zsb = p1sb.tile([128, T1], F32, tag='zsb')
                if t % 2 == 0:
                    nc.vector.tensor_copy(out=zsb[:], in_=zps[:])
                else:
                    nc.scalar.activation(zsb[:], zps[:],
                                         mybir.ActivationFunctionType.Copy)
                nc.sync.dma_start(out=z1tab[t*128:(t+1)*128, :], in_=zsb[:])

        # ---- edge phase (shared between layers)
        def edge_phase(layer, tab_lo, tab_hi, tabw, zcols, heads,
                       adst_ap, finalize):
            nmax = counts[layer]
            metarow_d = inp[f'metarow{layer}']
            tot = int(nmax.sum())

            with tc.tile_pool(name=f'ep{layer}', bufs=1) as ep, \
                 tc.tile_pool(name=f'em{layer}', bufs=3) as em, \
                 tc.tile_pool(name=f'eg{layer}', bufs=3) as eg, \
                 tc.tile_pool(name=f'ew{layer}', bufs=3) as ew, \
                 tc.tile_pool(name=f'el{layer}', bufs=2, space='PSUM') as el, \
                 tc.tile_pool(name=f'ea{layer}', bufs=2, space='PSUM') as ea:

                idx_lo = ep.tile([128, inp[f'idxlo{layer}'].shape[1]], I16)
                nc.sync.dma_start(out=idx_lo[:], in_=inp[f'idxlo{layer}'][:, :])
                idx_hi = ep.tile([128, inp[f'idxhi{layer}'].shape[1]], I16)
                nc.sync.dma_start(out=idx_hi[:], in_=inp[f'idxhi{layer}'][:, :])
                metacol = ep.tile([128, tot], F32)
                nc.sync.dma_start(out=metacol[:], in_=inp[f'metacol{layer}'][:, :])

                gtiles = [{}, {}]
                stream_pos = [0, 0]
                mtiles = {}
                chunk_idx = 0
                for i in range(cfg.BPC):
                    nblk = int(nmax[i, 0] + nmax[i, 1])
                    if nblk == 0:
                        continue
                    adst_blk = ew.tile([128, heads], F32, tag='adst')
                    nc.sync.dma_start(out=adst_blk[:], in_=adst_ap(i))
                    accps = ea.tile([128, zcols + heads], F32, space='PSUM',
                                    tag='acc')
                    done = 0
                    for g in range(2):
                        for _k in range(int(nmax[i, g])):
                            pos = stream_pos[g]
                            stream_pos[g] += 1
                            gi, j = divmod(pos, GG)
                            if gi not in gtiles[g]:
                                gt = eg.tile([128, GG, tabw], F32, tag=f'g{g}')
                                nc.gpsimd.dma_gather(
                                    out_ap=gt[:],
                                    in_ap=tab_lo if g == 0 else tab_hi,
                                    idxs_ap=(idx_lo if g == 0 else idx_hi)[
                                        :, gi*64:(gi+1)*64],
                                    num_idxs=GG * BLK, num_idxs_reg=GG * BLK,
                                    elem_size=tabw)
                                gtiles[g][gi] = gt
                                for old in [k_ for k_ in gtiles[g]
                                            if k_ < gi - 2]:
                                    del gtiles[g][old]
                            gt = gtiles[g][gi]

                            mg, mj = divmod(chunk_idx, MG)
                            if mg not in mtiles:
                                lo = mg * MG * BLK
                                hi = min(tot * BLK, lo + MG * BLK)
                                mrow = em.tile([1, MG * BLK], F32, tag='mrow')
                                nc.sync.dma_start(out=mrow[0:1, 0:hi-lo],
                                                  in_=metarow_d[0:1, lo:hi])
                                mb_ps = el.tile([128, MG * BLK], F32,
                                                space='PSUM', tag='mb')
                                nc.tensor.matmul(out=mb_ps[:, 0:hi-lo],
                                                 lhsT=ones_r[:],
                                                 rhs=mrow[0:1, 0:hi-lo],
                                                 start=True, stop=True)
                                mtiles[mg] = mb_ps
                                for old in [k_ for k_ in mtiles
                                            if k_ < mg - 1]:
                                    del mtiles[old]
                            mb_ps = mtiles[mg]

                            onehot = ew.tile([128, 128], F32, tag='oh')
                            nc.vector.tensor_scalar(
                                out=onehot[:], in0=iota_r[:],
                                scalar1=metacol[:, chunk_idx:chunk_idx+1],
                                scalar2=None, op0=mybir.AluOpType.is_equal)
                            onehotT = ew.tile([128, 128], F32, tag='ohT')
                            nc.vector.tensor_scalar(
                                out=onehotT[:],
                                in0=mb_ps[:, mj*BLK:(mj+1)*BLK],
                                scalar1=iota_c[:, 0:1],
                                scalar2=None, op0=mybir.AluOpType.is_equal)

                            lg_ps = el.tile([128, heads], F32, space='PSUM',
                                            tag='lg')
                            nc.tensor.matmul(out=lg_ps[:], lhsT=onehotT[:],
                                             rhs=adst_blk[:],
                                             start=True, stop=False)
                            nc.tensor.matmul(out=lg_ps[:], lhsT=ident[:],
                                             rhs=gt[:, j, zcols:zcols+heads],
                                             start=False, stop=True)
                            lg02 = ew.tile([128, heads], F32, tag='lg02')
                            nc.vector.tensor_scalar(
                                out=lg02[:], in0=lg_ps[:], scalar1=NEG,
                                scalar2=None, op0=mybir.AluOpType.mult)
                            lrl = ew.tile([128, heads], F32, tag='lrl')
                            nc.vector.tensor_tensor(
                                out=lrl[:], in0=lg_ps[:], in1=lg02[:],
                                op=mybir.AluOpType.max)
                            w_t = ew.tile([128, heads], F32, tag='wt')
                            nc.scalar.activation(w_t[:], lrl[:],
                                                 mybir.ActivationFunctionType.Exp)

                            smsg = ew.tile([128, zcols], F32, tag='smsg')
                            if heads > 1:
                                nc.vector.tensor_tensor(
                                    out=smsg[:], in0=gt[:, j, 0:zcols],
                                    in1=w_t[:, None, :].to_broadcast(
                                        [128, zcols // heads, heads]),
                                    op=mybir.AluOpType.mult)
                            else:
                                nc.vector.tensor_scalar(
                                    out=smsg[:], in0=gt[:, j, 0:zcols],
                                    scalar1=w_t[:, 0:1], scalar2=None,
                                    op0=mybir.AluOpType.mult)

                            nc.tensor.matmul(out=accps[:, 0:zcols],
                                             lhsT=onehot[:], rhs=smsg[:],
                                             start=(done == 0), stop=False)
                            nc.tensor.matmul(out=accps[:, zcols:zcols+heads],
                                             lhsT=onehot[:], rhs=w_t[:],
                                             start=False,
                                             stop=(done == nblk - 1))
                            done += 1
                            chunk_idx += 1
                    finalize(i, accps)

        # ---- L1 finalize: normalize + bias + elu -> h1loc
        def fin1(i, accps):
            den = sbw.tile([128, H], F32, tag='den')
            nc.vector.tensor_scalar(out=den[:], in0=accps[:, D1:D1+H],
                                    scalar1=1e-30, scalar2=None,
                                    op0=mybir.AluOpType.max)
            rec = sbw.tile([128, H], F32, tag='rec')
            nc.vector.reciprocal(out=rec[:], in_=den[:])
            h1t = sbw.tile([128, D1], F32, tag='h1t')
            nc.vector.tensor_tensor(
                out=h1t[:], in0=accps[:, 0:D1],
                in1=rec[:, None, :].to_broadcast([128, D1 // H, H]),
                op=mybir.AluOpType.mult)
            h1c = sbw.tile([128, D1], F32, tag='h1c')
            nc.vector.tensor_tensor(out=h1c[:], in0=h1t[:], in1=b1b[:],
                                    op=mybir.AluOpType.add)
            # elu(x) = exp(min(x,0)) - 1 + max(x,0)
            m0 = sbw.tile([128, D1], F32, tag='m0')
            nc.vector.tensor_scalar(out=m0[:], in0=h1c[:], scalar1=0.0,
                                    scalar2=None, op0=mybir.AluOpType.min)
            ex = sbw.tile([128, D1], F32, tag='ex')
            nc.scalar.activation(ex[:], m0[:], mybir.ActivationFunctionType.Exp)
            rl = sbw.tile([128, D1], F32, tag='rl')
            nc.vector.tensor_scalar(out=rl[:], in0=h1c[:], scalar1=0.0,
                                    scalar2=None, op0=mybir.AluOpType.max)
            h1f = sbw.tile([128, D1], F32, tag='h1f')
            nc.vector.tensor_tensor(out=h1f[:], in0=ex[:], in1=rl[:],
                                    op=mybir.AluOpType.add)
            h1o = sbw.tile([128, D1], F32, tag='h1o')
            nc.vector.tensor_scalar(out=h1o[:], in0=h1f[:], scalar1=-1.0,
                                    scalar2=None, op0=mybir.AluOpType.add)
            nc.sync.dma_start(out=h1loc[i*128:(i+1)*128, :], in_=h1o[:])

        edge_phase(1, z1tab[0:cfg.HALF, :], z1tab[cfg.HALF:, :], T1, D1, H,
                   lambda i: z1tab[i*128:(i+1)*128, D1+H:D1+2*H], fin1)

        # ---- P3: z2slice = [h1 @ W2p | asrc2 | adst2]
        with tc.tile_pool(name='p3sb', bufs=3) as p3sb, \
             tc.tile_pool(name='p3ps', bufs=2, space='PSUM') as p3ps:
            for t in range(cfg.BPC):
                h1tile = p3sb.tile([128, D1], F32, tag='h1')
                nc.sync.dma_start(out=h1tile[:], in_=h1loc[t*128:(t+1)*128, :])
                zps = p3ps.tile([128, T2], F32, space='PSUM', tag='z')
                for fh in range(FH):
                    tp = p3ps.tile([128, 128], F32, space='PSUM', tag='tp')
                    nc.tensor.transpose(out=tp[:],
                                        in_=h1tile[:, fh*128:(fh+1)*128],
                                        identity=ident[:])
                    h1T = p3sb.tile([128, 128], F32, tag='h1T')
                    if fh % 2 == 0:
                        nc.vector.tensor_copy(out=h1T[:], in_=tp[:])
                    else:
                        nc.scalar.activation(h1T[:], tp[:],
                                             mybir.ActivationFunctionType.Copy)
                    nc.tensor.matmul(out=zps[:], lhsT=h1T[:], rhs=W2e[fh][:],
                                     start=(fh == 0), stop=(fh == FH - 1))
                zsb = p3sb.tile([128, T2], F32, tag='zsb')
                nc.vector.tensor_copy(out=zsb[:], in_=zps[:])
                nc.sync.dma_start(out=z2slice[t*128:(t+1)*128, :], in_=zsb[:])

        # ---- P4: AllGather z2slice -> z2cat
        nc.gpsimd.collective_compute(
            'AllGather', mybir.AluOpType.bypass,
            replica_groups=[list(range(cfg.CORES))],
            ins=[z2slice.ap().opt()],
            outs=[z2cat.ap().opt()])

        # ---- L2 finalize: normalize + bias -> out
        def fin2(i, accps):
            den = sbw.tile([128, 1], F32, tag='den2')
            nc.vector.tensor_scalar(out=den[:], in0=accps[:, CLS:CLS+1],
                                    scalar1=1e-30, scalar2=None,
                                    op0=mybir.AluOpType.max)
            rec2 = sbw.tile([128, 1], F32, tag='rec2')
            nc.vector.reciprocal(out=rec2[:], in_=den[:])
            o1 = sbw.tile([128, CLS], F32, tag='o1')
            nc.vector.tensor_scalar(out=o1[:], in0=accps[:, 0:CLS],
                                    scalar1=rec2[:, 0:1], scalar2=None,
                                    op0=mybir.AluOpType.mult)
            o2 = sbw.tile([128, CLS], F32, tag='o2')
            nc.vector.tensor_tensor(out=o2[:], in0=o1[:], in1=b2b[:],
                                    op=mybir.AluOpType.add)
            nc.sync.dma_start(out=out_d[i*128:(i+1)*128, :], in_=o2[:])

        edge_phase(2, z2cat[0:cfg.HALF, :], z2cat[cfg.HALF:, :], T2, CLS, 1,
                   lambda i: z2slice[i*128:(i+1)*128, CLS+1:CLS+2], fin2)

    return nc


# ---------------------------------------------------------------- entry point

def run(inputs, cfg, sim=False):
    counts, per_core = host_prep(inputs['edge_index'], cfg)
    consts, xT_rots = host_weights(inputs, cfg)
    nc = build_gat(counts, cfg)
    in_maps = []
    for c in range(cfg.CORES):
        m = dict(consts)
        m['xT_rot'] = xT_rots[c]
        m.update(per_core[c])
        in_maps.append(m)
    if not sim:
        nc.compile()
    if sim:
        from concourse import bass_interp
        ms = bass_interp.MultiCoreSim(nc, cfg.CORES,
                                      num_workers=min(8, cfg.CORES))
        for c in range(cfg.CORES):
            for k, v in in_maps[c].items():
                ms.cores[c].tensor(k)[:] = v
        ms.simulate()
        outs = [np.array(ms.cores[c].mem_tensor('out')).reshape(cfg.SLICE, cfg.CLS)
                for c in range(cfg.CORES)]
    else:
        res = run_bass_kernel_spmd(nc, in_maps, core_ids=list(range(cfg.CORES)))
        outs = [np.asarray(res.results[c]['out']).reshape(cfg.SLICE, cfg.CLS)
                for c in range(cfg.CORES)]
    full = np.concatenate(outs, 0)
    return full[:cfg.N].astype(np.float32)


def kernel(**inputs):
    return run(inputs, FULL, sim=False)



# revision 22
# speedup vs baseline: 1.9475x; 1.9475x over previous
"""Trainium2 Bass kernel for 2-layer GAT (nn_GAT_84146999263862).

Strategy (8 NeuronCores, SPMD):
  - Nodes padded to NP=50176 = 8*49*128; core c owns node slice [c*6272,(c+1)*6272).
  - Edges (plus self-loops) are assigned to cores by dst slice. Per (layer,
    stream) edges are sorted by (dst block, dst_local) and PACKED contiguously
    into 128-edge chunks (chunks may span dst-block boundaries; per-block edge
    counts are padded to the max across cores so the block schedule is a
    shared static program).
  - Gather tables are bf16; per-edge rows fetched with gpsimd.dma_gather
    (int16 indices => two streams split at table row 32768).
  - Per chunk: is_equal one-hot matrices (bf16) map edges<->dst slots; PE
    matmuls (bf16) do adst expansion and segment reduction into fp32 PSUM.
    Leaky-relu+exp run on the Scalar engine (Prelu/Exp), batched 4 chunks per
    instruction. Softmax skips max-subtraction (logits are O(1)).
  - Layer-1 z table is built per-core in ROTATED node order with fp32r
    matmuls; one small bf16 AllGather shares z2 between layers.

Host does integer-only preprocessing (sorting/packing/index maps and pure data
movement like transposes); all floating-point math runs on device.
"""
import sys
import numpy as np

sys.path.insert(0, '/opt/trn_rl_repo')

import ml_dtypes
from contextlib import ExitStack
from concourse import bass, bacc, mybir, tile, library_config
from concourse.bass_utils import run_bass_kernel_spmd
from concourse.masks import make_identity
from concourse.tile import ScopedClock

BF16NP = ml_dtypes.bfloat16

# This walrus build rejects multi-wait TPB_CTRL instructions; split the Tile
# tail-drain's semaphore waits across single-wait drains.
_MAXW = 1


def _patched_drain_and_barrier(self, tick_clock, wait_clock):
    drain = self.nc.sync.drain()
    wait_clock.add_sem_waits(drain.ins,
                             ScopedClock({None: tick_clock.global_clock}))
    si = drain.ins.sync_info
    waits = list(si.on_wait)
    if len(waits) > _MAXW:
        si.on_wait = waits[:_MAXW]
        for k in range(_MAXW, len(waits), _MAXW):
            extra = self.nc.sync.drain()
            extra.ins.sync_info = mybir.SyncInfo(on_wait=waits[k:k+_MAXW],
                                                 on_update=[])
    self.nc.all_engine_barrier()
    popped = self.nc._tile_sem_poison_stack.pop()
    assert popped is self._sem_poison
    self.nc.clear_and_free_semaphores(list(self.sems.allocated().values()))
    self.nc.all_engine_barrier()


tile.TileContext._drain_and_barrier = _patched_drain_and_barrier

F32 = mybir.dt.float32
F32R = mybir.dt.float32r
BF16 = mybir.dt.bfloat16
I16 = mybir.dt.int16
AF = mybir.ActivationFunctionType
OP = mybir.AluOpType
NEG = 0.2
BLK = 128
GG = 8                     # chunks per dma_gather instruction (1024 indices)
CG = 4                     # chunks per processing group (lg/exp/msg batching)
SG = 4                     # segments per one-hot group


class Cfg:
    def __init__(self, N=50000, cores=8, bpc=49, half=32768,
                 f_in=256, heads=8, ch=32, cls_=32, n_edges=800000):
        self.N = N
        self.CORES = cores
        self.BPC = bpc
        self.SLICE = bpc * BLK
        self.NP = cores * self.SLICE
        self.HALF = half
        self.F_IN = f_in
        self.HEADS = heads
        self.CH = ch
        self.D1 = heads * ch
        self.CLS = cls_
        self.N_EDGES = n_edges
        self.T1W = 384              # z1 row: z(256)|asrc(8)|adst(8)|pad
        self.T2W = 128              # z2 row: z2(32)|one|asrc2|adst2|pad
        self.E1 = self.D1 + 2 * heads   # meaningful z1 cols
        self.E2 = cls_ + 3              # meaningful z2 cols
        assert self.T1W * 2 % 256 == 0 and self.T2W * 2 % 256 == 0
        assert self.HALF % BLK == 0
        assert self.NP - self.HALF <= 32767


FULL = Cfg()


# ---------------------------------------------------------------- host side

def _wrap16_groups(vals):
    """[L] ints (L multiple of 1024) -> [128, L//16] int16 (wrapped 16
    partitions, replicated x8, per 1024-index group)."""
    L = len(vals)
    ng = L // 1024
    arr = np.zeros((128, ng * 64), np.int16)
    for gi in range(ng):
        v = np.asarray(vals[gi*1024:(gi+1)*1024], np.int64).reshape(64, 16)
        blkv = v.T.astype(np.int16)
        for r in range(8):
            arr[r*16:(r+1)*16, gi*64:(gi+1)*64] = blkv
    return arr


def _build_schedule(mshared, bpc):
    """mshared: [2, BPC] padded per-(stream,block) edge counts (shared across
    cores). Returns per-stream: (nchunks, seg list) where each seg is
    (chunk j, block b, lo, hi) with [lo,hi) the edge range inside the chunk."""
    out = []
    for g in range(2):
        offs = np.concatenate([[0], np.cumsum(mshared[g])]).astype(np.int64)
        L = int(offs[-1])
        nchunks = -(-L // BLK)
        segs = []
        for b in range(bpc):
            lo, hi = int(offs[b]), int(offs[b + 1])
            if hi == lo:
                continue
            for j in range(lo // BLK, (hi - 1) // BLK + 1):
                s = max(lo, j * BLK) - j * BLK
                e = min(hi, (j + 1) * BLK) - j * BLK
                segs.append((j, b, s, e))
        out.append((nchunks, segs, offs))
    return out


def _merge_events(sched, cg):
    """Merge the two streams' chunk-groups by block progress and insert block
    finalize events. Returns a list of events:
      ('grp', g, j0, k, [segment-entries for those chunks])
      ('fin', b)
    Segment entries: (j, b, lo, hi, seg_global_idx)."""
    groups = []           # per stream: list of (j0, k, segs, min_blk, max_blk)
    for g in range(2):
        nchunks, segs, _ = sched[g]
        seg_by_chunk = {}
        for si, (j, b, lo, hi) in enumerate(segs):
            seg_by_chunk.setdefault(j, []).append((j, b, lo, hi, si))
        glist = []
        for j0 in range(0, nchunks, cg):
            k = min(cg, nchunks - j0)
            ss = []
            for j in range(j0, j0 + k):
                ss.extend(seg_by_chunk.get(j, []))
            if not ss:
                continue
            blks = [s[1] for s in ss]
            glist.append((j0, k, ss, min(blks), max(blks)))
        groups.append(glist)

    events = []
    p = [0, 0]
    finalized = set()
    all_blocks = sorted({s[1] for g in range(2) for s in sched[g][1]})

    def next_blk(g):
        return groups[g][p[g]][3] if p[g] < len(groups[g]) else 10**9

    while p[0] < len(groups[0]) or p[1] < len(groups[1]):
        g = 0 if next_blk(0) <= next_blk(1) else 1
        j0, k, ss, _, _ = groups[g][p[g]]
        events.append(('grp', g, j0, k, ss))
        p[g] += 1
        horizon = min(next_blk(0), next_blk(1))
        for b in all_blocks:
            if b < horizon and b not in finalized:
                events.append(('fin', b))
                finalized.add(b)
    for b in all_blocks:
        if b not in finalized:
            events.append(('fin', b))
            finalized.add(b)
    return events


def host_prep(edge_index, cfg):
    """Integer-only preprocessing. Returns (meta, per_core_arrays).

    meta: dict with per-layer shared schedules + array shapes.
    per_core: list of dicts of device input arrays."""
    src = np.asarray(edge_index[0], np.int64)
    dst = np.asarray(edge_index[1], np.int64)
    loops = np.arange(cfg.N, dtype=np.int64)
    src = np.concatenate([src, loops])
    dst = np.concatenate([dst, loops])

    core = dst // cfg.SLICE
    blk = (dst % cfg.SLICE) // BLK
    dlv = dst % BLK

    meta = {}
    per_core = [dict() for _ in range(cfg.CORES)]
    for layer in (1, 2):
        # per-core streams, sorted by (block, dst_local)
        streams = {}                       # (c, g) -> (idx array, dl array, blk array)
        n = np.zeros((cfg.CORES, 2, cfg.BPC), np.int64)
        for c in range(cfg.CORES):
            m_c = core == c
            s_c, d_c, b_c = src[m_c], dlv[m_c], blk[m_c]
            ids = ((s_c - cfg.SLICE * c) % cfg.NP) if layer == 1 else s_c
            g_c = (ids >= cfg.HALF).astype(np.int64)
            idv = ids - g_c * cfg.HALF
            for g in range(2):
                m = g_c == g
                order = np.lexsort((d_c[m], b_c[m]))
                streams[(c, g)] = (idv[m][order], d_c[m][order], b_c[m][order])
                for b in range(cfg.BPC):
                    n[c, g, b] = int((b_c[m] == b).sum())
        mshared = n.max(axis=0)            # [2, BPC]
        sched = _build_schedule(mshared, cfg.BPC)
        events = _merge_events(sched, CG)
        meta[layer] = {'sched': sched, 'events': events}

        for g in range(2):
            nchunks, segs, offs = sched[g]
            L = int(offs[-1])
            Lpad = -(-max(L, 1) // 1024) * 1024
            nseg = len(segs)
            for c in range(cfg.CORES):
                idv, dv, bv = streams[(c, g)]
                idx_stream = np.zeros(Lpad, np.int64)
                dl_stream = np.full(Lpad, 255, np.int64)
                # place each block's edges at its shared offset
                pos = 0
                for b in range(cfg.BPC):
                    cnt = int((bv == b).sum())
                    o = int(offs[b])
                    idx_stream[o:o+cnt] = idv[pos:pos+cnt]
                    dl_stream[o:o+cnt] = dv[pos:pos+cnt]
                    pos += cnt
                data = per_core[c]
                data[f'idx{layer}{g}'] = _wrap16_groups(idx_stream)
                mc = np.full((BLK, max(nseg, 1)), 255, np.float32)
                mr = np.full((1, max(nseg, 1) * BLK), 255, np.float32)
                for si, (j, b, lo, hi) in enumerate(segs):
                    colv = np.full(BLK, 255, np.int64)
                    colv[lo:hi] = dl_stream[j*BLK+lo:j*BLK+hi]
                    mc[:, si] = colv
                    mr[0, si*BLK:(si+1)*BLK] = colv
                # layer-1 metacol feeds a tensor-tensor is_equal (bf16);
                # layer-2 metacol feeds tensor_scalar scalar1 (must be f32)
                data[f'metacol{layer}{g}'] = (mc.astype(BF16NP) if layer == 1
                                              else mc)
                data[f'metarow{layer}{g}'] = mr.astype(BF16NP)
    return meta, per_core


def host_weights(inputs, cfg):
    """Weight/constant staging (reordering + transposes only, no math)."""
    W1 = np.asarray(inputs['W1'], np.float32)
    a_src1 = np.asarray(inputs['a_src1'], np.float32)
    a_dst1 = np.asarray(inputs['a_dst1'], np.float32)
    b1 = np.asarray(inputs['b1'], np.float32)
    W2 = np.asarray(inputs['W2'], np.float32)
    a_src2 = np.asarray(inputs['a_src2'], np.float32)
    a_dst2 = np.asarray(inputs['a_dst2'], np.float32)
    b2 = np.asarray(inputs['b2'], np.float32)

    H, C, D1 = cfg.HEADS, cfg.CH, cfg.D1
    perm = np.empty(D1, np.int64)
    for h in range(H):
        for c_ in range(C):
            perm[c_*H + h] = h*C + c_
    consts = {
        'W1cm': W1[:, perm].copy(), 'W1T': W1.T.copy(),
        'a_src1': a_src1, 'a_dst1': a_dst1, 'b1cm': b1[perm][None, :].copy(),
        'W2p': W2[perm, :].copy(), 'W2pT': W2[perm, :].T.copy(),
        'a_src2': a_src2, 'a_dst2': a_dst2, 'b2': b2[None, :].copy(),
        'iota_r': np.broadcast_to(np.arange(128, dtype=np.float32),
                                  (128, 128)).astype(BF16NP).copy(),
        'iota_c': np.arange(128, dtype=np.float32)[:, None].copy(),
        'ones_f': np.ones((1, 128), np.float32),
        'ones_b': np.ones((1, 128), np.float32).astype(BF16NP),
    }
    x = np.asarray(inputs['x'], np.float32)
    xpad = np.zeros((cfg.NP, cfg.F_IN), np.float32)
    xpad[:cfg.N] = x
    xT_rots = [np.roll(xpad, -cfg.SLICE * c, axis=0).T.copy()
               for c in range(cfg.CORES)]
    return consts, xT_rots


# ---------------------------------------------------------------- device side

def build_gat(meta, cfg, use_prelu=True):
    nc = bacc.Bacc()
    H, C, D1, CLS, F_IN = cfg.HEADS, cfg.CH, cfg.D1, cfg.CLS, cfg.F_IN
    T1, T2, E1, E2 = cfg.T1W, cfg.T2W, cfg.E1, cfg.E2
    FH = F_IN // 128
    D1H = D1 // 128

    inp = {}
    decls = [
        ('xT_rot', [F_IN, cfg.NP], F32),
        ('W1cm', [F_IN, D1], F32), ('W1T', [D1, F_IN], F32),
        ('a_src1', [H, C], F32), ('a_dst1', [H, C], F32),
        ('b1cm', [1, D1], F32),
        ('W2p', [D1, CLS], F32), ('W2pT', [CLS, D1], F32),
        ('a_src2', [1, CLS], F32), ('a_dst2', [1, CLS], F32),
        ('b2', [1, CLS], F32),
        ('iota_r', [128, 128], BF16), ('iota_c', [128, 1], F32),
        ('ones_f', [1, 128], F32), ('ones_b', [1, 128], BF16),
    ]
    for layer in (1, 2):
        for g in range(2):
            nchunks, segs, offs = meta[layer]['sched'][g]
            L = int(offs[-1])
            Lpad = -(-max(L, 1) // 1024) * 1024
            nseg = max(len(segs), 1)
            decls.append((f'idx{layer}{g}', [128, Lpad // 16], I16))
            decls.append((f'metacol{layer}{g}', [128, nseg],
                          BF16 if layer == 1 else F32))
            decls.append((f'metarow{layer}{g}', [1, nseg * BLK], BF16))
    for name, shape, dt in decls:
        inp[name] = nc.declare_dram_parameter(name, shape, dt, isOutput=False)

    out_d = nc.declare_dram_parameter('out', [cfg.SLICE, CLS], F32,
                                      isOutput=True)

    z1tab = nc.dram_tensor('z1tab', [cfg.NP, T1], BF16)
    z2slice = nc.dram_tensor('z2slice', [cfg.SLICE, T2], BF16)
    z2cat = nc.dram_tensor('z2cat', [cfg.NP, T2], BF16, addr_space='Shared')

    with tile.TileContext(nc) as tc, ExitStack() as ctx:
        sb = ctx.enter_context(tc.tile_pool(name='sb', bufs=1))
        sbw = ctx.enter_context(tc.tile_pool(name='sbw', bufs=2))

        nc.gpsimd.load_library(library_config.mlp)

        ident_b = sb.tile([128, 128], BF16)
        make_identity(nc, ident_b[:])
        iota_r = sb.tile([128, 128], BF16)
        nc.sync.dma_start(out=iota_r[:], in_=inp['iota_r'][:, :])
        iota_c = sb.tile([128, 1], F32)
        nc.sync.dma_start(out=iota_c[:], in_=inp['iota_c'][:, :])
        ones_f = sb.tile([1, 128], F32)
        nc.sync.dma_start(out=ones_f[:], in_=inp['ones_f'][:, :])
        ones_b = sb.tile([1, 128], BF16)
        nc.sync.dma_start(out=ones_b[:], in_=inp['ones_b'][:, :])

        # persistent weight tiles (bf16)
        W1e = [sb.tile([128, E1], BF16, tag=f'w1e{i}', name=f'W1e{i}')
               for i in range(FH)]
        W2e = [sb.tile([128, E2], BF16, tag=f'w2e{i}', name=f'W2e{i}')
               for i in range(D1H)]
        b1b = sb.tile([128, D1], BF16)
        b2b = sb.tile([128, CLS], F32)
        h1sb = [sb.tile([128, D1], BF16, tag=f'h1_{i}', name=f'h1_{i}')
                for i in range(cfg.BPC)]

        # ---- P0: weight staging
        with tc.tile_pool(name='p0sb', bufs=1) as p0sb, \
             tc.tile_pool(name='p0ps', bufs=1, space='PSUM') as p0ps:
            # W1_ext = [W1cm | W1@A_src | W1@A_dst]
            a1 = p0sb.tile([H, 2 * C], F32)
            nc.sync.dma_start(out=a1[:, 0:C], in_=inp['a_src1'][:, :])
            nc.sync.dma_start(out=a1[:, C:2*C], in_=inp['a_dst1'][:, :])
            identf = p0sb.tile([128, 128], F32)
            make_identity(nc, identf[:])
            a1T_ps = p0ps.tile([128, 128], F32, space='PSUM', tag='t')
            nc.tensor.transpose(out=a1T_ps[0:2*C, 0:H], in_=a1[:],
                                identity=identf[0:H, 0:H])
            a1T = p0sb.tile([2 * C, H], F32)
            nc.vector.tensor_copy(out=a1T[:], in_=a1T_ps[0:2*C, 0:H])
            A_bd = p0sb.tile([128, FH, 2 * H], F32)
            nc.vector.memset(A_bd[:], 0.0)
            for h in range(H):
                half, off = divmod(h * C, 128)
                nc.vector.tensor_copy(out=A_bd[off:off+C, half, h:h+1],
                                      in_=a1T[0:C, h:h+1])
                nc.vector.tensor_copy(out=A_bd[off:off+C, half, H+h:H+h+1],
                                      in_=a1T[C:2*C, h:h+1])
            w1t_sb = [p0sb.tile([128, F_IN], F32, tag=f'w1t{i}', name=f'w1t{i}')
                      for i in range(D1H)]
            for cc in range(D1H):
                nc.sync.dma_start(out=w1t_sb[cc][:],
                                  in_=inp['W1T'][cc*128:(cc+1)*128, :])
            w1a_ps = p0ps.tile([128, FH, 2 * H], F32, space='PSUM', tag='a')
            for fh in range(FH):
                for cc in range(D1H):
                    nc.tensor.matmul(out=w1a_ps[:, fh, :],
                                     lhsT=w1t_sb[cc][:, fh*128:(fh+1)*128],
                                     rhs=A_bd[:, cc, :],
                                     start=(cc == 0), stop=(cc == D1H - 1))
            for fh in range(FH):
                w1tmp = p0sb.tile([128, D1], F32, tag='w1tmp')
                nc.sync.dma_start(out=w1tmp[:],
                                  in_=inp['W1cm'][fh*128:(fh+1)*128, :])
                nc.vector.tensor_copy(out=W1e[fh][:, 0:D1], in_=w1tmp[:])
                nc.vector.tensor_copy(out=W1e[fh][:, D1:D1+2*H],
                                      in_=w1a_ps[:, fh, :])

            # W2_ext = [W2p | 0 | W2p@a_src2^T | W2p@a_dst2^T]  (bf16)
            a2 = p0sb.tile([2, CLS], F32)
            nc.sync.dma_start(out=a2[0:1, :], in_=inp['a_src2'][:, :])
            nc.sync.dma_start(out=a2[1:2, :], in_=inp['a_dst2'][:, :])
            a2T_ps = p0ps.tile([128, 128], F32, space='PSUM', tag='t')
            nc.tensor.transpose(out=a2T_ps[0:CLS, 0:2], in_=a2[:],
                                identity=identf[0:2, 0:2])
            a2T = p0sb.tile([CLS, 2], F32)
            nc.vector.tensor_copy(out=a2T[:], in_=a2T_ps[0:CLS, 0:2])
            w2t_sb = p0sb.tile([CLS, D1], F32)
            nc.sync.dma_start(out=w2t_sb[:], in_=inp['W2pT'][:, :])
            w2a_ps = p0ps.tile([128, D1H, 2], F32, space='PSUM', tag='a')
            for fh in range(D1H):
                nc.tensor.matmul(out=w2a_ps[:, fh, :],
                                 lhsT=w2t_sb[:, fh*128:(fh+1)*128],
                                 rhs=a2T[:], start=True, stop=True)
            for fh in range(D1H):
                w2tmp = p0sb.tile([128, CLS], F32, tag='w2tmp')
                nc.sync.dma_start(out=w2tmp[:],
                                  in_=inp['W2p'][fh*128:(fh+1)*128, :])
                nc.vector.memset(W2e[fh][:], 0.0)
                nc.vector.tensor_copy(out=W2e[fh][:, 0:CLS], in_=w2tmp[:])
                nc.vector.tensor_copy(out=W2e[fh][:, CLS+1:CLS+3],
                                      in_=w2a_ps[:, fh, :])

            # bias broadcast tiles
            b1_sb = p0sb.tile([1, D1], F32)
            nc.sync.dma_start(out=b1_sb[:], in_=inp['b1cm'][:, :])
            b1b_ps = p0ps.tile([128, D1], F32, space='PSUM', tag='b')
            nc.tensor.matmul(out=b1b_ps[:], lhsT=ones_f[:], rhs=b1_sb[:],
                             start=True, stop=True)
            nc.vector.tensor_copy(out=b1b[:], in_=b1b_ps[:])
            b2_sb = p0sb.tile([1, CLS], F32)
            nc.sync.dma_start(out=b2_sb[:], in_=inp['b2'][:, :])
            b2b_ps = p0ps.tile([128, CLS], F32, space='PSUM', tag='b')
            nc.tensor.matmul(out=b2b_ps[:], lhsT=ones_f[:], rhs=b2_sb[:],
                             start=True, stop=True)
            nc.vector.tensor_copy(out=b2b[:], in_=b2b_ps[:])

        # ---- P1: z1 table build (rotated order, bf16 matmuls)
        with tc.tile_pool(name='p1sb', bufs=3) as p1sb, \
             tc.tile_pool(name='p1ps', bufs=2, space='PSUM') as p1ps:
            for t in range(cfg.NP // 128):
                zps = p1ps.tile([128, E1], F32, space='PSUM')
                for fh in range(FH):
                    xt = p1sb.tile([128, 128], F32, tag='xt')
                    nc.sync.dma_start(
                        out=xt[:],
                        in_=inp['xT_rot'][fh*128:(fh+1)*128, t*128:(t+1)*128])
                    xtb = p1sb.tile([128, 128], BF16, tag='xtb')
                    if (t + fh) % 2 == 0:
                        nc.vector.tensor_copy(out=xtb[:], in_=xt[:])
                    else:
                        nc.scalar.activation(xtb[:], xt[:], AF.Copy)
                    nc.tensor.matmul(out=zps[:], lhsT=xtb[:], rhs=W1e[fh][:],
                                     start=(fh == 0), stop=(fh == FH - 1))
                zsb = p1sb.tile([128, T1], BF16, tag='zsb')
                if t % 2 == 0:
                    nc.vector.tensor_copy(out=zsb[:, 0:E1], in_=zps[:])
                else:
                    nc.scalar.activation(zsb[:, 0:E1], zps[:], AF.Copy)
                nc.vector.memset(zsb[:, E1:T1], 0.0)
                nc.sync.dma_start(out=z1tab[t*128:(t+1)*128, :], in_=zsb[:])

        # ---- edge phase (shared between layers)
        def edge_phase(layer, tab_lo, tab_hi, tabw, zc, heads, adst_ap, fin):
            events = meta[layer]['events']
            sched = meta[layer]['sched']

            with tc.tile_pool(name=f'eg{layer}', bufs=3) as eg, \
                 tc.tile_pool(name=f'eo{layer}', bufs=3) as eo, \
                 tc.tile_pool(name=f'em{layer}', bufs=2) as em, \
                 tc.tile_pool(name=f'ew{layer}', bufs=3) as ew, \
                 tc.tile_pool(name=f'emp{layer}', bufs=2, space='PSUM') as emp, \
                 tc.tile_pool(name=f'elp{layer}', bufs=2, space='PSUM') as elp, \
                 tc.tile_pool(name=f'eap{layer}', bufs=4, space='PSUM') as eap:

                idx_sb = {}
                metacol = {}
                for g in range(2):
                    dident = inp[f'idx{layer}{g}']
                    t_ = eg.tile([128, dident.shape[1]], I16, tag=f'idx{g}',
                                 name=f'idx{layer}{g}')
                    nc.sync.dma_start(out=t_[:], in_=dident[:, :])
                    idx_sb[g] = t_
                    mcd = inp[f'metacol{layer}{g}']
                    mt = eg.tile([128, mcd.shape[1]],
                                 BF16 if layer == 1 else F32, tag=f'mc{g}',
                                 name=f'mc{layer}{g}')
                    nc.sync.dma_start(out=mt[:], in_=mcd[:, :])
                    metacol[g] = mt

                gtiles = {}        # (g, gi) -> gather tile
                segtiles = {}      # (g, sgi) -> (onehot tile, ohT tile, nseg)
                accs = {}          # b -> (acc tile, started flag)
                acc_last = {}      # b -> (g, seg idx) of last contribution
                # find last seg per block for start/stop flags
                seg_count = {}
                for ev in events:
                    if ev[0] != 'grp':
                        continue
                    _, g, j0, k, ss = ev
                    for (j, b, lo, hi, si) in ss:
                        seg_count[b] = seg_count.get(b, 0) + 1
                seg_seen = {}

                def get_gtile(g, gi):
                    key = (g, gi)
                    if key not in gtiles:
                        gt = eg.tile([128, GG, tabw], BF16, tag=f'g{g}')
                        nc.gpsimd.dma_gather(
                            out_ap=gt[:],
                            in_ap=tab_lo if g == 0 else tab_hi,
                            idxs_ap=idx_sb[g][:, gi*64:(gi+1)*64],
                            num_idxs=GG * BLK, num_idxs_reg=GG * BLK,
                            elem_size=tabw)
                        gtiles[key] = gt
                        for old in [k_ for k_ in gtiles
                                    if k_[0] == g and k_[1] < gi - 1]:
                            del gtiles[old]
                    return gtiles[key]

                def get_segtiles(g, sgi):
                    """One-hot + one-hotT tiles for segment group sgi of
                    stream g (segments sgi*SG .. sgi*SG+ns-1)."""
                    key = (g, sgi)
                    if key not in segtiles:
                        nseg_all = len(sched[g][1])
                        s0 = sgi * SG
                        ns = min(SG, nseg_all - s0)
                        oh = None
                        if layer == 1:
                            # onehot [128, ns, 128] bf16
                            oh = eo.tile([128, SG, 128], BF16, tag=f'oh{g}')
                            nc.vector.tensor_tensor(
                                out=oh[:, 0:ns, :],
                                in0=iota_r[:, None, :].to_broadcast(
                                    [128, ns, 128]),
                                in1=metacol[g][:, s0:s0+ns, None].to_broadcast(
                                    [128, ns, 128]),
                                op=OP.is_equal)
                        # metarow broadcast -> PSUM -> bf16 SBUF -> ohT
                        mrow = em.tile([1, SG * BLK], BF16, tag=f'mr{g}')
                        nc.sync.dma_start(
                            out=mrow[0:1, 0:ns*BLK],
                            in_=inp[f'metarow{layer}{g}'][0:1,
                                                          s0*BLK:(s0+ns)*BLK])
                        mb_ps = emp.tile([128, SG * BLK], F32, space='PSUM',
                                         tag='mb')
                        nc.tensor.matmul(out=mb_ps[:, 0:ns*BLK],
                                         lhsT=ones_b[:],
                                         rhs=mrow[0:1, 0:ns*BLK],
                                         start=True, stop=True)
                        mb_sb = em.tile([128, SG * BLK], BF16, tag=f'mbs{g}')
                        nc.scalar.activation(mb_sb[:, 0:ns*BLK],
                                             mb_ps[:, 0:ns*BLK], AF.Copy)
                        ohT = eo.tile([128, SG * BLK], BF16, tag=f'ohT{g}')
                        nc.vector.tensor_scalar(
                            out=ohT[:, 0:ns*BLK], in0=mb_sb[:, 0:ns*BLK],
                            scalar1=iota_c[:, 0:1], scalar2=None,
                            op0=OP.is_equal)
                        segtiles[key] = (oh, ohT)
                        for old in [k_ for k_ in segtiles
                                    if k_[0] == g and k_[1] < sgi - 1]:
                            del segtiles[old]
                    return segtiles[key]

                adst_cache = {}

                def get_adst(b):
                    if b not in adst_cache:
                        t_ = ew.tile([128, heads], BF16, tag='adst')
                        nc.sync.dma_start(out=t_[:], in_=adst_ap(b))
                        adst_cache[b] = t_
                        for old in [k_ for k_ in adst_cache if k_ < b - 2]:
                            del adst_cache[old]
                    return adst_cache[b]

                for ev in events:
                    if ev[0] == 'fin':
                        b = ev[1]
                        if b in accs:
                            fin(b, accs[b])
                            del accs[b]
                        continue
                    _, g, j0, k, ss = ev
                    # ensure gather tiles
                    for j in range(j0, j0 + k):
                        get_gtile(g, j // GG)
                    gt = gtiles[(g, j0 // GG)]
                    # assumes CG divides GG: one chunk-group never spans
                    # two gather tiles
                    # lg psum [128, k*heads]
                    lg_ps = elp.tile([128, CG * heads], F32, space='PSUM',
                                     tag='lg')
                    segs_by_chunk = {}
                    for (j, b, lo, hi, si) in ss:
                        segs_by_chunk.setdefault(j, []).append((b, si))
                    for j in range(j0, j0 + k):
                        jl = j - j0
                        joff = j % GG
                        lst = segs_by_chunk.get(j, [])
                        for q, (b, si) in enumerate(lst):
                            oh, ohT = get_segtiles(g, si // SG)
                            sl = si % SG
                            nc.tensor.matmul(
                                out=lg_ps[:, jl*heads:(jl+1)*heads],
                                lhsT=ohT[:, sl*BLK:(sl+1)*BLK],
                                rhs=get_adst(b)[:],
                                start=(q == 0), stop=False)
                        # + asrc (gathered) via identity matmul
                        nc.tensor.matmul(
                            out=lg_ps[:, jl*heads:(jl+1)*heads],
                            lhsT=ident_b[:],
                            rhs=gt[:, joff, zc+1:zc+1+heads] if layer == 2
                            else gt[:, joff, zc:zc+heads],
                            start=(len(lst) == 0), stop=True)
                    # leaky relu + exp on scalar engine
                    lgs = ew.tile([128, CG * heads], BF16, tag='lgs')
                    if use_prelu:
                        nc.scalar.activation(lgs[:, 0:k*heads],
                                             lg_ps[:, 0:k*heads],
                                             AF.Prelu, alpha=NEG)
                    else:
                        lg02 = ew.tile([128, CG * heads], BF16, tag='lg02')
                        nc.vector.tensor_scalar(
                            out=lg02[:, 0:k*heads], in0=lg_ps[:, 0:k*heads],
                            scalar1=NEG, scalar2=None, op0=OP.mult)
                        nc.vector.tensor_tensor(
                            out=lgs[:, 0:k*heads], in0=lg_ps[:, 0:k*heads],
                            in1=lg02[:, 0:k*heads], op=OP.max)
                    if layer == 1:
                        msg = ew.tile([128, CG, zc + heads], BF16, tag='msg')
                        nc.scalar.activation(
                            msg[:, 0:k, zc:zc+heads],
                            lgs[:, 0:k*heads], AF.Exp)
                        for j in range(j0, j0 + k):
                            jl = j - j0
                            joff = j % GG
                            nc.vector.tensor_tensor(
                                out=msg[:, jl, 0:zc],
                                in0=gt[:, joff, 0:zc],
                                in1=msg[:, jl, zc:zc+heads][:, None, :]
                                .to_broadcast([128, zc // heads, heads]),
                                op=OP.mult)
                    else:
                        wt = ew.tile([128, CG], F32, tag='wt')
                        nc.scalar.activation(wt[:, 0:k], lgs[:, 0:k], AF.Exp)
                    # accumulate into per-block acc psum
                    for j in range(j0, j0 + k):
                        jl = j - j0
                        joff = j % GG
                        for (b, si) in segs_by_chunk.get(j, []):
                            oh, ohT = get_segtiles(g, si // SG)
                            sl = si % SG
                            if b not in accs:
                                accs[b] = eap.tile(
                                    [128, zc + heads], F32, space='PSUM',
                                    tag='acc', name=f'acc{layer}_{b}')
                            seg_seen[b] = seg_seen.get(b, 0) + 1
                            first = seg_seen[b] == 1
                            last = seg_seen[b] == seg_count[b]
                            if layer == 1:
                                nc.tensor.matmul(
                                    out=accs[b][:],
                                    lhsT=oh[:, sl, :],
                                    rhs=msg[:, jl, :],
                                    start=first, stop=last,
                                    skip_group_check=True)
                            else:
                                ohw = ew.tile([128, 128], BF16, tag='ohw')
                                nc.vector.tensor_scalar(
                                    out=ohw[:], in0=iota_r[:],
                                    scalar1=metacol[g][:, si:si+1],
                                    scalar2=wt[:, jl:jl+1],
                                    op0=OP.is_equal, op1=OP.mult)
                                nc.tensor.matmul(
                                    out=accs[b][:],
                                    lhsT=ohw[:],
                                    rhs=gt[:, joff, 0:zc+1],
                                    start=first, stop=last,
                                    skip_group_check=True)

        # ---- L1 finalize: normalize + bias + elu -> h1sb (SBUF, bf16)
        def fin1(b, accps):
            den = sbw.tile([128, H], F32, tag='den')
            nc.vector.tensor_scalar(out=den[:], in0=accps[:, D1:D1+H],
                                    scalar1=1e-30, scalar2=None, op0=OP.max)
            rec = sbw.tile([128, H], F32, tag='rec')
            nc.vector.reciprocal(out=rec[:], in_=den[:])
            h1t = sbw.tile([128, D1], BF16, tag='h1t')
            nc.vector.tensor_tensor(
                out=h1t[:], in0=accps[:, 0:D1],
                in1=rec[:, None, :].to_broadcast([128, D1 // H, H]),
                op=OP.mult)
            h1c = sbw.tile([128, D1], BF16, tag='h1c')
            nc.vector.tensor_tensor(out=h1c[:], in0=h1t[:], in1=b1b[:],
                                    op=OP.add)
            # elu(x) = exp(min(x,0)) - 1 + max(x,0)
            m0 = sbw.tile([128, D1], BF16, tag='m0')
            nc.vector.tensor_scalar(out=m0[:], in0=h1c[:], scalar1=0.0,
                                    scalar2=None, op0=OP.min)
            ex = sbw.tile([128, D1], BF16, tag='ex')
            nc.scalar.activation(ex[:], m0[:], AF.Exp)
            rl = sbw.tile([128, D1], BF16, tag='rl')
            nc.vector.tensor_scalar(out=rl[:], in0=h1c[:], scalar1=0.0,
                                    scalar2=-1.0, op0=OP.max, op1=OP.add)
            nc.vector.tensor_tensor(out=h1sb[b][:], in0=ex[:], in1=rl[:],
                                    op=OP.add)

        edge_phase(1, z1tab[0:cfg.HALF, :], z1tab[cfg.HALF:, :], T1, D1, H,
                   lambda b: z1tab[b*128:(b+1)*128, D1+H:D1+2*H], fin1)

        # ---- P3: z2slice = [h1 @ W2p | 1 | asrc2 | adst2]  (bf16)
        with tc.tile_pool(name='p3sb', bufs=3) as p3sb, \
             tc.tile_pool(name='p3ps', bufs=2, space='PSUM') as p3ps:
            for t in range(cfg.BPC):
                zps = p3ps.tile([128, E2], F32, space='PSUM', tag='z')
                for fh in range(D1H):
                    tp = p3ps.tile([128, 128], BF16, space='PSUM', tag='tp')
                    nc.tensor.transpose(out=tp[:],
                                        in_=h1sb[t][:, fh*128:(fh+1)*128],
                                        identity=ident_b[:])
                    h1T = p3sb.tile([128, 128], BF16, tag='h1T')
                    if fh % 2 == 0:
                        nc.vector.tensor_copy(out=h1T[:], in_=tp[:])
                    else:
                        nc.scalar.activation(h1T[:], tp[:], AF.Copy)
                    nc.tensor.matmul(out=zps[:], lhsT=h1T[:], rhs=W2e[fh][:],
                                     start=(fh == 0), stop=(fh == D1H - 1))
                zsb = p3sb.tile([128, T2], BF16, tag='zsb')
                nc.vector.tensor_copy(out=zsb[:, 0:E2], in_=zps[:])
                nc.vector.memset(zsb[:, CLS:CLS+1], 1.0)
                nc.vector.memset(zsb[:, E2:T2], 0.0)
                nc.sync.dma_start(out=z2slice[t*128:(t+1)*128, :], in_=zsb[:])

        # ---- P4: AllGather z2slice -> z2cat
        nc.gpsimd.collective_compute(
            'AllGather', OP.bypass,
            replica_groups=[list(range(cfg.CORES))],
            ins=[z2slice.ap().opt()],
            outs=[z2cat.ap().opt()])

        # ---- L2 finalize: normalize + bias -> out
        def fin2(b, accps):
            den = sbw.tile([128, 1], F32, tag='den2')
            nc.vector.tensor_scalar(out=den[:], in0=accps[:, CLS:CLS+1],
                                    scalar1=1e-30, scalar2=None, op0=OP.max)
            rec2 = sbw.tile([128, 1], F32, tag='rec2')
            nc.vector.reciprocal(out=rec2[:], in_=den[:])
            o1 = sbw.tile([128, CLS], F32, tag='o1')
            nc.vector.tensor_scalar(out=o1[:], in0=accps[:, 0:CLS],
                                    scalar1=rec2[:, 0:1], scalar2=None,
                                    op0=OP.mult)
            o2 = sbw.tile([128, CLS], F32, tag='o2')
            nc.vector.tensor_tensor(out=o2[:], in0=o1[:], in1=b2b[:],
                                    op=OP.add)
            nc.sync.dma_start(out=out_d[b*128:(b+1)*128, :], in_=o2[:])

        edge_phase(2, z2cat[0:cfg.HALF, :], z2cat[cfg.HALF:, :], T2, CLS, 1,
                   lambda b: z2slice[b*128:(b+1)*128, CLS+2:CLS+3], fin2)

    return nc


# ---------------------------------------------------------------- entry point

def run(inputs, cfg, sim=False, use_prelu=True):
    meta, per_core = host_prep(inputs['edge_index'], cfg)
    consts, xT_rots = host_weights(inputs, cfg)
    nc = build_gat(meta, cfg, use_prelu=use_prelu)
    in_maps = []
    for c in range(cfg.CORES):
        m = dict(consts)
        m['xT_rot'] = xT_rots[c]
        m.update(per_core[c])
        in_maps.append(m)
    if not sim:
        nc.compile()
    if sim:
        from concourse import bass_interp
        ms = bass_interp.MultiCoreSim(nc, cfg.CORES,
                                      num_workers=min(8, cfg.CORES))
        for c in range(cfg.CORES):
            for k, v in in_maps[c].items():
                ms.cores[c].tensor(k)[:] = v
        ms.simulate()
        outs = [np.array(ms.cores[c].mem_tensor('out')).reshape(cfg.SLICE,
                                                                cfg.CLS)
                for c in range(cfg.CORES)]
    else:
        res = run_bass_kernel_spmd(nc, in_maps, core_ids=list(range(cfg.CORES)))
        outs = [np.asarray(res.results[c]['out']).reshape(cfg.SLICE, cfg.CLS)
                for c in range(cfg.CORES)]
    full = np.concatenate(outs, 0)
    return full[:cfg.N].astype(np.float32)


def kernel(**inputs):
    return run(inputs, FULL, sim=False)
